# revision 32
# baseline (speedup 1.0000x reference)
"""Trainium2 Bass kernel for causal multi-head attention with RoPE.

Problem: B=2, T=2048, C=2048, H=16, D=128.
Sharding over 8 NeuronCores: batch (2) x head-group (4 heads each); the host
sums the 4 per-head-group partials per batch and adds bo' = bo + bv @ Wo.T
(the v-bias commutes through softmax since rows sum to 1).

v3.1 design notes:
- All matmuls bf16 (fp8 is numerically fatal here: the softmax is peaked
  enough that fp8 x alone gives 4% output error).
- x staged as 64 [128,512] tiles (per k-slab x T-chunk) so the first matmul
  only waits on 2MB of DMA, not the full 8MB.
- Q/K projections: h-outer, k-outer, chunk-inner with 4 live chunk-psums;
  the stationary weight tile is reused across 4 matmuls so LDWEIGHTS has 4x
  the slack to hide (measured 46ns/mm exposure otherwise).
- V projection split: tt0..7 in phase A, tt8..11 / tt12..15 + O-proj blocks
  used as PE spacers between attention groups so the scalar engine's exp
  (the attention pacing engine) gets catch-up windows and the PE never
  idles long enough to re-throttle the HAM clock gate.
- RoPE off the PE: rotate-half via SBUF->SBUF DMA partition swap (sign
  folded into the sin table), 2 DVE mults + GPSIMD add.
- Max-free softmax, scores pre-transposed ST=[tk,tq]; exp batched [128,1024]
  per j-pair; ones-vector matmul row-sums in PSUM.
- PSUM budget exactly 8 banks in each phase: A: qk ring 8x[128,512];
  B: ST pairs 2x2 + av 1 + ones 1 + op/V-spacer ring 2.
"""

import math
import sys

import numpy as np

for _p in ("/opt/trn_rl_repo", "/root/.axon_site/_ro/trn_rl_repo"):
    if _p not in sys.path:
        sys.path.append(_p)

import ml_dtypes

import concourse.bacc as bacc
import concourse.bass as bass
import concourse.mybir as mybir
import concourse.tile as tile
from contextlib import ExitStack

F32 = mybir.dt.float32
BF = mybir.dt.bfloat16
AF = mybir.ActivationFunctionType
ALU = mybir.AluOpType

B, T, C = 2, 2048, 2048
H, D = 16, 128
THETA = 10000.0
NEG = -1e9

N_CORES = 8
GROUPS = 4          # head groups (other shard axis is batch)
HPC = H // GROUPS   # heads per core
KT = C // 128       # contraction k-slabs
GW = 512            # T-group width
NG = T // GW        # attention groups
NT = T // 128       # 128-wide t-tiles
NCH = T // GW       # x column chunks
BF_NP = ml_dtypes.bfloat16


def build_core_nc(debug=False):
    nc = bacc.Bacc(None, target_bir_lowering=False, debug=debug)

    xT = nc.dram_tensor("xT", [C, T], BF, kind="ExternalInput")
    wqT = nc.dram_tensor("wqT", [C, HPC * 128], BF, kind="ExternalInput")
    wkT = nc.dram_tensor("wkT", [C, HPC * 128], BF, kind="ExternalInput")
    wvT = nc.dram_tensor("wvT", [C, HPC * 128], BF, kind="ExternalInput")
    woT = nc.dram_tensor("woT", [HPC * 128, C], BF, kind="ExternalInput")
    bq = nc.dram_tensor("bq", [HPC * 128], F32, kind="ExternalInput")
    bk = nc.dram_tensor("bk", [HPC * 128], F32, kind="ExternalInput")
    cosT = nc.dram_tensor("cosT", [128, T], BF, kind="ExternalInput")
    sinM = nc.dram_tensor("sinM", [128, T], BF, kind="ExternalInput")
    maskT = nc.dram_tensor("maskT", [128, 128], F32, kind="ExternalInput")
    ones = nc.dram_tensor("ones", [128, 1], BF, kind="ExternalInput")
    out = nc.dram_tensor("out", [T, C], F32, kind="ExternalOutput")

    with tile.TileContext(nc) as tc, ExitStack() as top:
        const = top.enter_context(tc.tile_pool(name="const", bufs=1))
        big = top.enter_context(tc.tile_pool(name="big", bufs=1))

        bq_sb = const.tile([128, HPC], F32, name="bq_sb")
        nc.sync.dma_start(bq_sb[:], bq.rearrange("(h d) -> d h", d=128))
        bk_sb = const.tile([128, HPC], F32, name="bk_sb")
        nc.sync.dma_start(bk_sb[:], bk.rearrange("(h d) -> d h", d=128))
        mask_sb = const.tile([128, 128], F32, name="mask_sb")
        nc.sync.dma_start(mask_sb[:], maskT[:, :])
        ones_sb = const.tile([128, 1], BF, name="ones_sb")
        nc.sync.dma_start(ones_sb[:], ones[:, :])

        # Phase-A-scoped pool opens early: weight tiles die with phase A.
        pa = ExitStack()
        paw = pa.enter_context(tc.tile_pool(name="paw", bufs=1))

        # Stationary weight tiles for K then Q: 8 x [128, KT, 128].
        wqk = []
        for i, (wdram, nm) in enumerate(((wkT, "k"), (wqT, "q"))):
            for h in range(HPC):
                wt = paw.tile([128, KT, 128], BF, name=f"w_{nm}{h}")
                nc.sync.dma_start(
                    wt[:],
                    wdram[:, h * 128 : (h + 1) * 128].rearrange(
                        "(ko ki) n -> ki ko n", ki=128
                    ),
                )
                wqk.append(wt)
                if i == 0 and h == 0:
                    # x chunk 0 right after the first weight tile
                    x_t = [[None] * NCH for _ in range(KT)]
                    for k in range(KT):
                        x_t[k][0] = big.tile([128, GW], BF, name=f"x_{k}_0")
                        nc.sync.dma_start(
                            x_t[k][0][:],
                            xT[k * 128 : (k + 1) * 128, 0:GW],
                        )
                    cos_sb = const.tile([128, T], BF, name="cos_sb")
                    nc.sync.dma_start(cos_sb[:], cosT[:, :])
                    sin_sb = const.tile([128, T], BF, name="sin_sb")
                    nc.sync.dma_start(sin_sb[:], sinM[:, :])
        wv_sb = big.tile([128, KT, HPC * 128], BF, name="wv_sb")
        nc.sync.dma_start(
            wv_sb[:], wvT.rearrange("(ko ki) n -> ki ko n", ki=128)
        )
        for ch in range(1, NCH):
            for k in range(KT):
                x_t[k][ch] = big.tile([128, GW], BF, name=f"x_{k}_{ch}")
                nc.sync.dma_start(
                    x_t[k][ch][:],
                    xT[k * 128 : (k + 1) * 128, ch * GW : (ch + 1) * GW],
                )
        wo_sb = big.tile([128, HPC, C], BF, name="wo_sb")
        nc.sync.dma_start(
            wo_sb[:], woT.rearrange("(ho hi) c -> hi ho c", hi=128)
        )
        qT = big.tile([128, HPC, T], BF, name="qT")
        kTt = big.tile([128, HPC, T], BF, name="kTt")
        v_sb = big.tile([128, NT, HPC * 128], BF, name="v_sb")

        def vproj_tt(tt, pool, tag):
            ps = pool.tile([128, HPC * 128], F32, tag=tag, name=f"vps_{tt}")
            for k in range(KT):
                nc.tensor.matmul(
                    ps[:],
                    x_t[k][tt // 4][:, (tt % 4) * 128 : (tt % 4 + 1) * 128],
                    wv_sb[:, k, :],
                    start=(k == 0),
                    stop=(k == KT - 1),
                )
            nc.scalar.copy(v_sb[:, tt, :], ps[:])

        # ---- Phase A: Q/K projections + V tt0..7 ----
        with pa:
            qkp = pa.enter_context(tc.tile_pool(name="qkp", bufs=8, space="PSUM"))
            raw = pa.enter_context(tc.tile_pool(name="raw", bufs=4))

            def rope_evac(ps, ch, bias_sb, dstT, h):
                cols = slice(ch * GW, (ch + 1) * GW)
                rawt = raw.tile([128, GW], BF, tag="raw")
                nc.scalar.activation(
                    rawt[:], ps[:], AF.Identity, bias=bias_sb[:, h : h + 1]
                )
                swap = raw.tile([128, GW], BF, tag="swap")
                nc.sync.dma_start(swap[0:64, :], rawt[64:128, :])
                nc.sync.dma_start(swap[64:128, :], rawt[0:64, :])
                t1 = raw.tile([128, GW], BF, tag="t1")
                nc.vector.tensor_tensor(t1[:], rawt[:], cos_sb[:, cols], ALU.mult)
                t2 = raw.tile([128, GW], BF, tag="t2")
                nc.vector.tensor_tensor(t2[:], swap[:], sin_sb[:, cols], ALU.mult)
                nc.gpsimd.tensor_tensor(dstT[:, h, cols], t1[:], t2[:], ALU.add)

            # consumption order tracks DMA arrival order (x streams at
            # ~200-350 GB/s): ch0 singles, ch1 singles, then (ch2,ch3) pairs
            def qk_block(chs):
                for i, (bias_sb, dstT) in enumerate(((bk_sb, kTt), (bq_sb, qT))):
                    for h in range(HPC):
                        wt = wqk[i * HPC + h]
                        psums = [
                            qkp.tile([128, GW], F32, tag="qk",
                                     name=f"qk_{i}_{h}_{ch}")
                            for ch in chs
                        ]
                        for k in range(KT):
                            for ci, ch in enumerate(chs):
                                nc.tensor.matmul(
                                    psums[ci][:],
                                    wt[:, k, :],
                                    x_t[k][ch][:],
                                    start=(k == 0),
                                    stop=(k == KT - 1),
                                )
                        for ci, ch in enumerate(chs):
                            rope_evac(psums[ci], ch, bias_sb, dstT, h)

            qk_block([0])
            qk_block([1, 2, 3])

            for tt in range(8):
                vproj_tt(tt, qkp, "qk")

        # ---- Phase B: attention + V tt8..15 + output projection ----
        with ExitStack() as pb:
            stp = pb.enter_context(tc.tile_pool(name="stp", bufs=2, space="PSUM"))
            avp = pb.enter_context(tc.tile_pool(name="avp", bufs=1, space="PSUM"))
            onp = pb.enter_context(tc.tile_pool(name="onp", bufs=1, space="PSUM"))
            opp = pb.enter_context(tc.tile_pool(name="opp", bufs=2, space="PSUM"))
            ptp = pb.enter_context(tc.tile_pool(name="ptp", bufs=3))
            smp = pb.enter_context(tc.tile_pool(name="smp", bufs=1))
            nrm = pb.enter_context(tc.tile_pool(name="nrm", bufs=2))
            avs = pb.enter_context(tc.tile_pool(name="avs", bufs=2))
            att = pb.enter_context(tc.tile_pool(name="att", bufs=3))
            outp = pb.enter_context(tc.tile_pool(name="outp", bufs=2))

            def attn_head(g, h, att_g, filler=None):
                nblocks = 4 * g + 4
                npairs = nblocks // 2
                av = avp.tile([128, GW], F32, tag="av")
                on = onp.tile([1, GW], F32, tag="on")
                gcol0 = g * GW

                def emit_pair(jp, g=g, h=h):
                    st2 = stp.tile([128, 2 * GW], F32, tag="st")
                    pt2 = ptp.tile([128, 2 * GW], BF, tag="pt")
                    c0s = []
                    for jl in (0, 1):
                        j = 2 * jp + jl
                        di = j - 4 * g
                        c0 = di * 128 if di >= 0 else 0
                        c0s.append(c0)
                        nc.tensor.matmul(
                            st2[:, jl * GW + c0 : (jl + 1) * GW],
                            kTt[:, h, j * 128 : (j + 1) * 128],
                            qT[:, h, gcol0 + c0 : gcol0 + GW],
                            start=True,
                            stop=True,
                        )
                        if di >= 0:
                            nc.vector.tensor_tensor(
                                st2[:, jl * GW + c0 : jl * GW + c0 + 128],
                                st2[:, jl * GW + c0 : jl * GW + c0 + 128],
                                mask_sb[:],
                                ALU.add,
                            )
                    if c0s[1] == 0:
                        nc.scalar.activation(pt2[:], st2[:], AF.Exp)
                    else:
                        nc.scalar.activation(
                            pt2[:, c0s[0] : 2 * GW],
                            st2[:, c0s[0] : 2 * GW],
                            AF.Exp,
                        )
                    return pt2, c0s

                def consume(jp, pt2, c0s, h=h):
                    for jl in (0, 1):
                        j = 2 * jp + jl
                        c0 = c0s[jl]
                        sl = slice(jl * GW + c0, (jl + 1) * GW)
                        nc.tensor.matmul(
                            av[:, c0:GW],
                            v_sb[:, j, h * 128 : (h + 1) * 128],
                            pt2[:, sl],
                            start=(j == 0),
                            stop=(j == nblocks - 1),
                        )
                        nc.tensor.matmul(
                            on[0:1, c0:GW],
                            ones_sb[:],
                            pt2[:, sl],
                            start=(j == 0),
                            stop=(j == nblocks - 1),
                        )

                pend = []
                for jp in range(npairs):
                    pend.append((jp, *emit_pair(jp)))
                    if filler is not None:
                        filler()
                    if len(pend) > 1:
                        consume(*pend.pop(0))
                for item in pend:
                    consume(*item)
                if filler is not None:
                    filler()

                av_sb = avs.tile([128, GW], F32, tag="avsb")
                nc.vector.tensor_scalar_mul(av_sb[:], av[:], 1.0)
                on_sb = smp.tile([1, GW], F32, tag="onsb")
                nc.vector.tensor_scalar_mul(on_sb[0:1, :], on[0:1, :], 1.0)
                ri1 = smp.tile([1, GW], F32, tag="ri1")
                nc.vector.reciprocal_approx_fast(ri1[0:1, :], on_sb[0:1, :])
                ri = nrm.tile([128, GW], F32, tag="ri")
                nc.gpsimd.partition_broadcast(ri[:], ri1[0:1, :])
                nc.vector.tensor_tensor(
                    att_g[:, h, :], av_sb[:], ri[:], ALU.mult
                )

            def oproj_chunk(g, att_g, idx, act_evac=False):
                tloc, cch = idx // 4, idx % 4
                tt = g * 4 + tloc
                op = opp.tile([128, GW], F32, tag="op")
                for kc in range(HPC):
                    nc.tensor.matmul(
                        op[:],
                        att_g[:, kc, tloc * 128 : (tloc + 1) * 128],
                        wo_sb[:, kc, cch * GW : (cch + 1) * GW],
                        start=(kc == 0),
                        stop=(kc == HPC - 1),
                    )
                osb = outp.tile([128, GW], F32, tag="osb")
                # during attention interleave, keep ACT free for exp: DVE evac
                if act_evac:
                    nc.scalar.copy(osb[:], op[:])
                else:
                    nc.vector.tensor_scalar_mul(osb[:], op[:], 1.0)
                nc.sync.dma_start(
                    out[tt * 128 : (tt + 1) * 128, cch * GW : (cch + 1) * GW],
                    osb[:],
                )

            def vproj_gen(tts):
                for tt in tts:
                    ps = opp.tile([128, HPC * 128], F32, tag="op",
                                  name=f"vps_{tt}")
                    for k in range(KT):
                        nc.tensor.matmul(
                            ps[:],
                            x_t[k][tt // 4][:, (tt % 4) * 128 : (tt % 4 + 1) * 128],
                            wv_sb[:, k, :],
                            start=(k == 0),
                            stop=(k == KT - 1),
                        )
                        if k % 4 == 3:
                            yield
                    nc.vector.tensor_scalar_mul(v_sb[:, tt, :], ps[:], 1.0)

            def oproj_gen(g, att_g, lo=0, hi=16):
                for idx in range(lo, hi):
                    oproj_chunk(g, att_g, idx)
                    yield

            def chain_gens(*gens):
                for gen in gens:
                    for _ in gen:
                        yield

            class Filler:
                def __init__(self, gen):
                    self.gen = gen

                def __call__(self, n=1):
                    for _ in range(n):
                        try:
                            next(self.gen)
                        except StopIteration:
                            return

                def drain(self):
                    for _ in self.gen:
                        pass

            att_tiles = {}

            def attn_group(g, filler=None):
                att_g = att.tile(
                    [128, HPC, GW], BF, tag="att", name=f"att_{g}"
                )
                att_tiles[g] = att_g
                for h in range(HPC):
                    attn_head(g, h, att_g, filler)
                if filler is not None:
                    filler.drain()

            # block PE spacers between attention groups; G3's heads
            # interleave with O-proj half-blocks so exp always has PE cover
            def oproj_blk(g, lo, hi):
                for idx in range(lo, hi):
                    oproj_chunk(g, att_tiles[g], idx,
                                act_evac=(idx % 2 == 1))

            attn_group(0)
            for tt in range(8, 12):
                vproj_tt(tt, opp, "op")
            attn_group(1)
            for tt in range(12, 16):
                vproj_tt(tt, opp, "op")
            attn_group(2)
            oproj_blk(0, 0, 16)
            att_g3 = att.tile([128, HPC, GW], BF, tag="att", name="att_3")
            att_tiles[3] = att_g3
            attn_head(3, 0, att_g3)
            oproj_blk(1, 0, 8)
            attn_head(3, 1, att_g3)
            oproj_blk(1, 8, 16)
            attn_head(3, 2, att_g3)
            oproj_blk(2, 0, 8)
            attn_head(3, 3, att_g3)
            oproj_blk(2, 8, 16)
            oproj_blk(3, 0, 16)

    nc.compile()
    return nc


def _rope_tables(T_, theta=THETA):
    inv = 1.0 / (theta ** (np.arange(0, D, 2, dtype=np.float64) / D))
    t = np.arange(T_, dtype=np.float64)
    fr = np.outer(t, inv)
    emb = np.concatenate([fr, fr], axis=1)
    return (
        np.cos(emb).T.astype(np.float32).copy(),
        np.sin(emb).T.astype(np.float32).copy(),
    )


def _maskT():
    tk = np.arange(128)[:, None]
    c = np.arange(128)[None, :]
    return np.where(c >= tk, 0.0, NEG).astype(np.float32)


def prep_inputs(x, Wq, bq, Wk, bk, Wv, bv, Wo, bo):
    scale = 1.0 / math.sqrt(D)
    cosT, sinT = _rope_tables(T)
    sinMv = sinT.copy()
    sinMv[: D // 2] = -sinMv[: D // 2]
    maskT = _maskT()
    ones = np.ones((128, 1), dtype=BF_NP)
    cosT = cosT.astype(BF_NP)
    sinMv = sinMv.astype(BF_NP)
    xT = [np.ascontiguousarray(x[b].T).astype(BF_NP) for b in range(B)]
    in_maps = []
    for c in range(N_CORES):
        b, g = c // GROUPS, c % GROUPS
        rows = slice(g * HPC * D, (g + 1) * HPC * D)
        in_maps.append(
            {
                "xT": xT[b],
                "wqT": np.ascontiguousarray((Wq[rows] * scale).T).astype(BF_NP),
                "wkT": np.ascontiguousarray(Wk[rows].T).astype(BF_NP),
                "wvT": np.ascontiguousarray(Wv[rows].T).astype(BF_NP),
                "woT": np.ascontiguousarray(Wo[:, rows].T).astype(BF_NP),
                "bq": np.ascontiguousarray(bq[rows] * scale).astype(np.float32),
                "bk": np.ascontiguousarray(bk[rows]).astype(np.float32),
                "cosT": cosT,
                "sinM": sinMv,
                "maskT": maskT,
                "ones": ones,
            }
        )
    bo_eff = (bo + bv @ Wo.T).astype(np.float32)
    return in_maps, bo_eff


_NC_CACHE = {}


def get_nc():
    if "nc" not in _NC_CACHE:
        _NC_CACHE["nc"] = build_core_nc()
    return _NC_CACHE["nc"]


def kernel(x, Wq, bq, Wk, bk, Wv, bv, Wo, bo):
    x = np.asarray(x, dtype=np.float32)
    args = [np.asarray(a, dtype=np.float32) for a in (Wq, bq, Wk, bk, Wv, bv, Wo, bo)]
    in_maps, bo_eff = prep_inputs(x, *args)
    nc = get_nc()

    from concourse.bass_utils import run_bass_kernel_spmd

    res = run_bass_kernel_spmd(nc, in_maps, core_ids=list(range(N_CORES))).results

    out = np.empty((B, T, C), dtype=np.float32)
    for b in range(B):
        acc_ = res[b * GROUPS]["out"].astype(np.float32).copy()
        for g in range(1, GROUPS):
            acc_ += res[b * GROUPS + g]["out"]
        out[b] = acc_ + bo_eff
    return out
